# revision 2
# baseline (speedup 1.0000x reference)
"""Bass/Trainium2 kernel for nn_CalAttn (masked softmax attention, returns (out, attn)).

Sharding: B*H = 48 (batch, head) pairs -> 8 cores, 6 heads each; core c takes
batch b = c//2, heads h0 = 6*(c%2) .. h0+5. Each core holds full S so the SxS
score block is local (head parallelism per the sharding hint).

Device computes, per head, in the [k, q] orientation (k on partitions):
    sT   = K^T-contraction matmul (f32r) accumulated with a bf16
           identity-matmul that adds maskbias (0 / -2^33) into PSUM
    PT   = exp(0.125 * sT)            (ACT, one pass, f32r out; masked -> 0)
    attnT_unnorm = PT                 (DMA'd straight to DRAM)
    [out_unnorm | Z] = [V | 1]^T @ PT (PE, f32r, row 64 = masked row-sums)
Host then normalizes by Z and transposes attnT -> attn. This keeps every
engine to a single pass over the S x S block; the ~100MB/core attn write is
the roofline.
"""

import os
import sys
from contextlib import ExitStack

sys.path.insert(0, "/opt/trn_rl_repo")

import numpy as np
import ml_dtypes

import concourse.bass as bass
import concourse.tile as tile
from concourse import bacc, mybir
from concourse._compat import with_exitstack
from concourse.bass_utils import run_bass_kernel_spmd
from concourse.masks import make_identity

B, H, S, D = 4, 12, 2048, 64
NCORES = 8
HPC = (B * H) // NCORES          # heads per core = 6
PAIRS = HPC // 2                 # head-pairs per core = 3
P = 128
KT = S // P                      # k tiles per head = 16
QC = 2                           # q chunks per k tile (1024 wide each)
QW = S // QC                     # q chunk width = 1024
MASKBIAS = -(2.0 ** 33)

_CACHED_NC = None
LAST_RESULT = None               # BassKernelResults from the most recent run


@with_exitstack
def _attn_kernel(ctx: ExitStack, tc: tile.TileContext, ins, outs):
    nc = tc.nc
    in_qt, in_kt, in_va, in_mb = ins
    out_attnt, out_oz = outs

    f32 = mybir.dt.float32
    f32r = mybir.dt.float32r
    bf16 = mybir.dt.bfloat16

    const_pool = ctx.enter_context(tc.tile_pool(name="const", bufs=1))
    qk_pool = ctx.enter_context(tc.tile_pool(name="qk", bufs=1))
    mb_pool = ctx.enter_context(tc.tile_pool(name="mb", bufs=1))
    va_pool = ctx.enter_context(tc.tile_pool(name="va", bufs=1))
    pt_pool = ctx.enter_context(tc.tile_pool(name="pt", bufs=3))
    ob_pool = ctx.enter_context(tc.tile_pool(name="ob", bufs=2))
    ps_pool = ctx.enter_context(tc.tile_pool(name="ps", bufs=2, space="PSUM"))
    po_pool = ctx.enter_context(tc.tile_pool(name="po", bufs=1, space="PSUM"))

    ident = const_pool.tile([P, P], bf16)
    make_identity(nc, ident)

    # Resident inputs. qT/kT head-pairs [128, S] (f32r via SWDGE cast-DMA),
    # mask bias tiles [128, S] bf16, V-augmented [128, KT, D+1] f32r per head.
    t_qt, t_kt = [], []
    for p in range(PAIRS):
        tq = qk_pool.tile([P, S], f32r, tag=f"qt{p}")
        nc.gpsimd.dma_start(tq[:], in_qt[p])
        t_qt.append(tq)
        tk = qk_pool.tile([P, S], f32r, tag=f"kt{p}")
        nc.gpsimd.dma_start(tk[:], in_kt[p])
        t_kt.append(tk)
    t_va = []
    for h in range(HPC):
        tv = va_pool.tile([P, KT, D + 1], f32r, tag=f"va{h}")
        nc.gpsimd.dma_start(tv[:], in_va[h])
        t_va.append(tv)
    t_mb = []
    for t in range(KT):
        tm = mb_pool.tile([P, S], bf16, tag=f"mb{t}")
        nc.sync.dma_start(tm[:], in_mb[t])
        t_mb.append(tm)

    for h in range(HPC):
        pair, half = divmod(h, 2)
        row0 = D * half
        kt_h = t_kt[pair][row0:row0 + D]     # [64, S]
        qt_h = t_qt[pair][row0:row0 + D]     # [64, S]

        # accumulator [V|1]^T @ PT for this head: [65, S] psum (4 banks)
        po = po_pool.tile([D + 1, S], f32, tag="po")

        for t in range(KT):
            pt_tile = pt_pool.tile([P, S], f32r, tag="pt")
            for qc in range(QC):
                ps = ps_pool.tile([P, QW], f32, tag="ps")
                # scores^T: two 512-wide f32r matmuls, then bf16 identity
                # matmul accumulates maskbias into the same psum region
                for s2 in range(QW // 512):
                    q0 = qc * QW + s2 * 512
                    nc.tensor.matmul(
                        ps[:, s2 * 512:(s2 + 1) * 512],
                        kt_h[:, t * P:(t + 1) * P],
                        qt_h[:, q0:q0 + 512],
                        start=True, stop=False,
                    )
                for s2 in range(QW // 512):
                    q0 = qc * QW + s2 * 512
                    nc.tensor.matmul(
                        ps[:, s2 * 512:(s2 + 1) * 512],
                        ident[:],
                        t_mb[t][:, q0:q0 + 512],
                        start=False, stop=True,
                    )
                # exp(0.125 * sT); masked entries underflow to exactly 0
                nc.scalar.activation(
                    pt_tile[:, qc * QW:(qc + 1) * QW],
                    ps[:],
                    mybir.ActivationFunctionType.Exp,
                    scale=0.125,
                )
            # unnormalized attn^T block straight to DRAM (1 MB)
            nc.sync.dma_start(out_attnt[h, t], pt_tile[:])
            # [out_un | Z] accumulation over k tiles
            for s4 in range(S // 512):
                nc.tensor.matmul(
                    po[:, s4 * 512:(s4 + 1) * 512],
                    t_va[h][:, t, :],
                    pt_tile[:, s4 * 512:(s4 + 1) * 512],
                    start=(t == 0), stop=(t == KT - 1),
                )

        ob = ob_pool.tile([D + 1, S], f32, tag="ob")
        nc.vector.tensor_copy(ob[:], po[:])
        nc.sync.dma_start(out_oz[h], ob[:])


def _build_nc():
    nc = bacc.Bacc("TRN2", target_bir_lowering=False, debug=False,
                   num_devices=NCORES)
    f32 = mybir.dt.float32
    in_qt = nc.dram_tensor("qt", [PAIRS, P, S], f32, kind="ExternalInput").ap()
    in_kt = nc.dram_tensor("kt", [PAIRS, P, S], f32, kind="ExternalInput").ap()
    in_va = nc.dram_tensor("va", [HPC, P, KT, D + 1], f32,
                           kind="ExternalInput").ap()
    in_mb = nc.dram_tensor("mb", [KT, P, S], mybir.dt.bfloat16,
                           kind="ExternalInput").ap()
    out_attnt = nc.dram_tensor("attnt", [HPC, KT, P, S], mybir.dt.float32r,
                               kind="ExternalOutput").ap()
    out_oz = nc.dram_tensor("oz", [HPC, D + 1, S], f32,
                            kind="ExternalOutput").ap()
    with tile.TileContext(nc) as tc:
        _attn_kernel(tc, (in_qt, in_kt, in_va, in_mb), (out_attnt, out_oz))
    nc.compile()
    return nc


def _host_shard(q, k, v, attn_pad_mask):
    """Build the 8 per-core input dicts."""
    in_maps = []
    ones = np.ones((S, 1), dtype=np.float32)
    for c in range(NCORES):
        b, h0 = c // 2, HPC * (c % 2)
        qb = q[b, h0:h0 + HPC]                       # [6, S, D]
        kb = k[b, h0:h0 + HPC]
        qt = np.ascontiguousarray(
            qb.transpose(0, 2, 1).reshape(PAIRS, 2 * D, S))
        kt = np.ascontiguousarray(
            kb.transpose(0, 2, 1).reshape(PAIRS, 2 * D, S))
        va = np.empty((HPC, P, KT, D + 1), dtype=np.float32)
        for h in range(HPC):
            aug = np.concatenate([v[b, h0 + h], ones], axis=1)  # [S, 65]
            va[h] = aug.reshape(KT, P, D + 1).transpose(1, 0, 2)
        mbT = attn_pad_mask[b].T                     # [k, q] bool
        mb = (mbT.astype(np.float32) * np.float32(MASKBIAS)).astype(
            ml_dtypes.bfloat16).reshape(KT, P, S)
        in_maps.append({"qt": qt, "kt": kt, "va": va, "mb": mb})
    return in_maps


def kernel(q, k, v, attn_pad_mask):
    global _CACHED_NC, LAST_RESULT
    q = np.asarray(q, dtype=np.float32)
    k = np.asarray(k, dtype=np.float32)
    v = np.asarray(v, dtype=np.float32)
    attn_pad_mask = np.asarray(attn_pad_mask).astype(bool)

    if _CACHED_NC is None:
        _CACHED_NC = _build_nc()
    nc = _CACHED_NC

    in_maps = _host_shard(q, k, v, attn_pad_mask)
    trace = bool(int(os.environ.get("ATTN_TRACE", "0")))
    res = run_bass_kernel_spmd(
        nc, in_maps, core_ids=list(range(NCORES)), trace=trace,
    )
    LAST_RESULT = res

    out = np.empty((B, H, S, D), dtype=np.float32)
    attn = np.empty((B, H, S, S), dtype=np.float32)
    for c in range(NCORES):
        b, h0 = c // 2, HPC * (c % 2)
        r = res.results[c]
        attnt = r["attnt"].reshape(HPC, S, S)        # [h, k, q]
        oz = r["oz"]                                 # [h, 65, q]
        for h in range(HPC):
            z = oz[h, D]                             # [q]
            z = np.where(z == 0.0, np.float32(1.0), z)
            rz = (np.float32(1.0) / z).astype(np.float32)
            out[b, h0 + h] = (oz[h, :D] * rz[None, :]).T
            attn[b, h0 + h] = (attnt[h] * rz[None, :]).T
    return out, attn
